# revision 17
# baseline (speedup 1.0000x reference)
"""FFT depthwise conv == direct 7x7 circular depthwise conv, on 8 TRN2 cores.

out[b,i,j,c] = sum_{u,v} wf[c,u,v] * x[b,(i+u-3)%H,(j+v-3)%W,c],  wf = kernel[:, ::-1, ::-1]

Banded-matmul scheme: image ROWS live on SBUF partitions, so one matmul with a
7-diagonal banded stationary matrix covers all 7 row-taps (u) at once; the 7
column-taps (v) become 7 PSUM-accumulated matmuls whose rhs is the same tile
shifted by v columns.  49 taps in 7 matmuls instead of 49.

Sharding: channels (192/8 = 24 per core), so all 8 images stream through each
banded weight while it is stationary (weight-load overhead amortized 8x).

Per (channel, row-half): input tile [118 rows x (8 img x 230 cols)] bf16; for
v in 0..6: one matmul per image-pair q (PSUM bank q, [112 x 448] f32,
start=(v==0), stop=(v==6)).  lhsT[p, m] = wf[c, p-m, v] (7-diag band).
PSUM -> bf16 SBUF evac split across ScalarE/VectorE, one 401KB output DMA per
(c, half).  Host pre-builds the circularly-padded row-major tiles and banded
weights; host also reassembles the final (B,H,W,C) output.
"""

import os
import sys

for _p in ("/opt/trn_rl_repo", "/root/.axon_site/_ro/trn_rl_repo"):
    if os.path.isdir(_p) and _p not in sys.path:
        sys.path.insert(0, _p)

import numpy as np

import concourse.bacc as bacc
import concourse.bass as bass
import concourse.mybir as mybir
from concourse.bass_utils import run_bass_kernel_spmd
from concourse.tile import TileContext

F32 = mybir.dt.float32
BF16 = mybir.dt.bfloat16

B, H, W, C, K = 8, 224, 224, 192, 7
NCORES = 8
CPC = C // NCORES        # 24 channels per core
PAD = K // 2             # 3
HALFR = H // 2           # 112 output rows per half
PROWS = HALFR + 2 * PAD  # 118 input rows actually used per half-tile
TROWS = 128              # tile partition rows (16-multiple so the DMA
                         # descriptor balancer sprays all 16 SDMA engines;
                         # 118 partitions degenerate to a 2-engine split)
WCOLS = 128              # banded lhsT padded to 128x128 so walrus enables
                         # FWL (fast weight load, 2 bf16/cycle) -- the
                         # per-matmul LDWEIGHTS then hides under the stream
NCOL = W + 2 * PAD       # 230 padded cols
NQ = 4                   # image pairs per (c, half): 4 x 2 = 8 images
QCOLS = 2 * W            # 448 psum cols per pair
PREFETCH = 3


def build_nc():
    nc = bacc.Bacc()
    x_d = nc.declare_dram_parameter("x", [CPC, 2, TROWS, B, NCOL], BF16, isOutput=False)
    w_d = nc.declare_dram_parameter("w", [TROWS, CPC, K, WCOLS], BF16, isOutput=False)
    out_d = nc.declare_dram_parameter("out", [CPC, 2, HALFR, NQ, QCOLS], BF16, isOutput=True)

    mult = mybir.AluOpType.mult

    with TileContext(nc) as tc:
        with (
            tc.tile_pool(name="wp", bufs=1) as wpool,
            tc.tile_pool(name="xp", bufs=PREFETCH + 2) as xpool,
            tc.tile_pool(name="op", bufs=3) as opool,
            tc.tile_pool(name="pp", bufs=8, space="PSUM") as ppool,
        ):
            wsb = wpool.tile([TROWS, CPC, K, WCOLS], BF16)

            units = [(c, h) for c in range(CPC) for h in range(2)]
            pending = {}

            def issue_x(i, split=False):
                c, h = units[i]
                xt = xpool.tile([TROWS, B, NCOL], BF16, name=f"xt{c}_{h}", tag="xt")
                if split:
                    # halve time-to-data by using both HWDGE queues
                    nc.sync.dma_start(out=xt[0:64], in_=x_d[c, h, 0:64])
                    nc.scalar.dma_start(out=xt[64:TROWS], in_=x_d[c, h, 64:TROWS])
                else:
                    nc.sync.dma_start(out=xt[:], in_=x_d[c, h])
                pending[i] = xt

            def issue_w(c):
                # per-channel weight DMAs, issued lazily ~2 channels ahead so
                # they neither stall compute nor head-block the scalar queue
                # (where output DMAs also live)
                nc.scalar.dma_start(out=wsb[:, c : c + 1], in_=w_d[:, c : c + 1])

            issue_x(0, split=True)
            issue_w(0)
            issue_w(1)
            for j in range(1, PREFETCH):
                issue_x(j)

            for i, (c, h) in enumerate(units):
                if i + PREFETCH < len(units):
                    issue_x(i + PREFETCH)
                if h == 0 and c + 2 < CPC:
                    issue_w(c + 2)
                xt = pending.pop(i)
                pss = []
                for q in range(NQ):
                    ps = ppool.tile([TROWS, QCOLS], F32, name=f"ps{c}_{h}_{q}", tag="ps")
                    pss.append(ps)
                for v in range(K):
                    wap = wsb[:, c, v, :]
                    for q in range(NQ):
                        nc.tensor.matmul(
                            pss[q][:],
                            wap,
                            xt[:, 2 * q : 2 * q + 2, v : v + W],
                            start=(v == 0),
                            stop=(v == K - 1),
                        )
                ot = opool.tile([HALFR, NQ, QCOLS], BF16, name=f"ot{c}_{h}", tag="ot")
                for q in range(NQ):
                    if q % 2 == 0:
                        nc.scalar.copy(out=ot[:, q, :], in_=pss[q][0:HALFR, :])
                    else:
                        nc.vector.tensor_scalar(ot[:, q, :], pss[q][0:HALFR, :], 1.0, None, mult)
                if i >= len(units) - 2:
                    # tail: split across both queues to shorten the epilogue
                    nc.scalar.dma_start(out=out_d[c, h, 0:56], in_=ot[0:56])
                    nc.sync.dma_start(out=out_d[c, h, 56:HALFR], in_=ot[56:HALFR])
                else:
                    eng = nc.scalar if i % 2 == 0 else nc.sync
                    eng.dma_start(out=out_d[c, h], in_=ot[:])
    return nc


def _host_pack_x(x):
    """x (B,H,W,C) f32 -> per-core [CPC, 2, TROWS, B, NCOL] bf16."""
    import ml_dtypes

    xt = np.transpose(x, (3, 1, 2, 0))  # (C, H, W, B)
    xt = np.concatenate([xt[:, :, -PAD:, :], xt, xt[:, :, :PAD, :]], axis=2)  # (C,H,230,B)
    halves = []
    for h in range(2):
        rows = (np.arange(TROWS) + h * HALFR - PAD) % H
        th = xt[:, rows]                       # (C, 128, 230, B)
        halves.append(np.transpose(th, (0, 1, 3, 2)))  # (C, 128, B, 230)
    xp = np.stack(halves, axis=1).astype(ml_dtypes.bfloat16)  # (C, 2, 128, B, 230)
    return [np.ascontiguousarray(xp[k * CPC : (k + 1) * CPC]) for k in range(NCORES)]


def _host_pack_w(kernel):
    """kernel (C,K,K) -> per-core banded lhsT [PROWS, CPC, K, HALFR] bf16.

    lhsT[p, cl, v, m] = wf[c0+cl, p-m, v] for 0 <= p-m < 7, else 0.
    """
    import ml_dtypes

    wf = np.ascontiguousarray(kernel[:, ::-1, ::-1]).astype(np.float32)  # (C, K, K)
    blobs = []
    m_idx = np.arange(HALFR)
    for k in range(NCORES):
        warr = np.zeros((TROWS, CPC, K, WCOLS), dtype=np.float32)
        wc = wf[k * CPC : (k + 1) * CPC]  # (24, 7, 7)
        for u in range(K):
            # warr[m+u, :, v, m] = wc[:, u, v]
            warr[m_idx + u, :, :, m_idx] = wc[:, u, :]
        blobs.append(warr.astype(ml_dtypes.bfloat16))
    return blobs


_NC_CACHE = {}


def _get_nc():
    if "nc" not in _NC_CACHE:
        nc = build_nc()
        nc.finalize()
        _NC_CACHE["nc"] = nc
    return _NC_CACHE["nc"]


def run(x, kernel, trace=False, **kw):
    assert x.shape == (B, H, W, C) and kernel.shape == (C, K, K)
    nc = _get_nc()
    xs = _host_pack_x(np.asarray(x).astype(np.float32))
    ws = _host_pack_w(np.asarray(kernel))
    in_maps = [{"x": xs[k], "w": ws[k]} for k in range(NCORES)]
    res = run_bass_kernel_spmd(nc, in_maps, list(range(NCORES)), trace=trace, **kw)
    # out blob [CPC, 2, HALFR, NQ, 448] -> (B, H, W, CPC) per core
    parts = []
    for k in range(NCORES):
        o = np.asarray(res.results[k]["out"]).astype(np.float32)
        o = o.reshape(CPC, 2, HALFR, B, W)          # (c, h, m, img, j)
        o = np.transpose(o, (3, 1, 2, 4, 0))        # (img, h, m, j, c)
        parts.append(o.reshape(B, H, W, CPC))
    out = np.concatenate(parts, axis=3)
    return np.ascontiguousarray(out), res


def kernel(x, kernel):
    out, _ = run(np.asarray(x), np.asarray(kernel))
    return out


# revision 19
# speedup vs baseline: 1.0076x; 1.0076x over previous
"""FFT depthwise conv == direct 7x7 circular depthwise conv, on 8 TRN2 cores.

out[b,i,j,c] = sum_{u,v} wf[c,u,v] * x[b,(i+u-3)%H,(j+v-3)%W,c],  wf = kernel[:, ::-1, ::-1]

Banded-matmul scheme: image ROWS live on SBUF partitions, so one matmul with a
7-diagonal banded stationary matrix covers all 7 row-taps (u) at once; the 7
column-taps (v) become 7 PSUM-accumulated matmuls whose rhs is the same tile
shifted by v columns.  49 taps in 7 matmuls instead of 49.

Sharding: channels (192/8 = 24 per core), so all 8 images stream through each
banded weight while it is stationary (weight-load overhead amortized 8x).

Per (channel, row-half): input tile [118 rows x (8 img x 230 cols)] bf16; for
v in 0..6: one matmul per image-pair q (PSUM bank q, [112 x 448] f32,
start=(v==0), stop=(v==6)).  lhsT[p, m] = wf[c, p-m, v] (7-diag band).
PSUM -> bf16 SBUF evac split across ScalarE/VectorE, one 401KB output DMA per
(c, half).  Host pre-builds the circularly-padded row-major tiles and banded
weights; host also reassembles the final (B,H,W,C) output.
"""

import os
import sys

for _p in ("/opt/trn_rl_repo", "/root/.axon_site/_ro/trn_rl_repo"):
    if os.path.isdir(_p) and _p not in sys.path:
        sys.path.insert(0, _p)

import numpy as np

import concourse.bacc as bacc
import concourse.bass as bass
import concourse.mybir as mybir
from concourse.bass_utils import run_bass_kernel_spmd
from concourse.tile import TileContext

F32 = mybir.dt.float32
BF16 = mybir.dt.bfloat16

B, H, W, C, K = 8, 224, 224, 192, 7
NCORES = 8
CPC = C // NCORES        # 24 channels per core
PAD = K // 2             # 3
HALFR = H // 2           # 112 output rows per half
PROWS = HALFR + 2 * PAD  # 118 input rows actually used per half-tile
TROWS = 128              # tile partition rows (16-multiple so the DMA
                         # descriptor balancer sprays all 16 SDMA engines;
                         # 118 partitions degenerate to a 2-engine split)
WCOLS = 128              # banded lhsT padded to 128x128 so walrus enables
                         # FWL (fast weight load, 2 bf16/cycle) -- the
                         # per-matmul LDWEIGHTS then hides under the stream
NCOL = W + 2 * PAD       # 230 padded cols
NQ = 4                   # image pairs per (c, half): 4 x 2 = 8 images
QCOLS = 2 * W            # 448 psum cols per pair
PREFETCH = 3


def build_nc():
    nc = bacc.Bacc()
    x_d = nc.declare_dram_parameter("x", [CPC, 2, TROWS, B, NCOL], BF16, isOutput=False)
    w_d = nc.declare_dram_parameter("w", [TROWS, CPC, K, WCOLS], BF16, isOutput=False)
    out_d = nc.declare_dram_parameter("out", [CPC, 2, HALFR, NQ, QCOLS], BF16, isOutput=True)

    mult = mybir.AluOpType.mult

    with TileContext(nc) as tc:
        with (
            tc.tile_pool(name="wp", bufs=1) as wpool,
            tc.tile_pool(name="xp", bufs=PREFETCH + 2) as xpool,
            tc.tile_pool(name="op", bufs=3) as opool,
            tc.tile_pool(name="pp", bufs=8, space="PSUM") as ppool,
        ):
            wsb = wpool.tile([TROWS, CPC, K, WCOLS], BF16)

            units = [(c, h) for c in range(CPC) for h in range(2)]
            pending = {}

            def issue_x(i):
                c, h = units[i]
                xt = xpool.tile([TROWS, B, NCOL], BF16, name=f"xt{c}_{h}", tag="xt")
                nc.sync.dma_start(out=xt[:], in_=x_d[c, h])
                pending[i] = xt

            def issue_w(c):
                # per-channel weight DMAs on the scalar queue (which carries
                # no activation-table load -- evac is DVE-only -- so these
                # start immediately after the prologue), issued lazily ~2
                # channels ahead of use
                nc.scalar.dma_start(out=wsb[:, c : c + 1], in_=w_d[:, c : c + 1])

            issue_w(0)
            issue_x(0)
            issue_w(1)
            for j in range(1, PREFETCH):
                issue_x(j)

            for i, (c, h) in enumerate(units):
                if i + PREFETCH < len(units):
                    issue_x(i + PREFETCH)
                if h == 0 and c + 2 < CPC:
                    issue_w(c + 2)
                xt = pending.pop(i)
                pss = []
                for q in range(NQ):
                    ps = ppool.tile([TROWS, QCOLS], F32, name=f"ps{c}_{h}_{q}", tag="ps")
                    pss.append(ps)
                for v in range(K):
                    wap = wsb[:, c, v, :]
                    for q in range(NQ):
                        nc.tensor.matmul(
                            pss[q][:],
                            wap,
                            xt[:, 2 * q : 2 * q + 2, v : v + W],
                            start=(v == 0),
                            stop=(v == K - 1),
                        )
                ot = opool.tile([HALFR, NQ, QCOLS], BF16, name=f"ot{c}_{h}", tag="ot")
                # evac on DVE only: using ScalarE would pull in a ~2.5us
                # InstLoadActFuncSet that head-blocks the scalar DMA queue
                for q in range(NQ):
                    nc.vector.tensor_scalar(ot[:, q, :], pss[q][0:HALFR, :], 1.0, None, mult)
                if i >= len(units) - 2:
                    # tail: split across both queues to shorten the epilogue
                    nc.scalar.dma_start(out=out_d[c, h, 0:56], in_=ot[0:56])
                    nc.sync.dma_start(out=out_d[c, h, 56:HALFR], in_=ot[56:HALFR])
                else:
                    eng = nc.scalar if i % 2 == 0 else nc.sync
                    eng.dma_start(out=out_d[c, h], in_=ot[:])
    return nc


def _host_pack_x(x):
    """x (B,H,W,C) f32 -> per-core [CPC, 2, TROWS, B, NCOL] bf16."""
    import ml_dtypes

    xt = np.transpose(x, (3, 1, 2, 0))  # (C, H, W, B)
    xt = np.concatenate([xt[:, :, -PAD:, :], xt, xt[:, :, :PAD, :]], axis=2)  # (C,H,230,B)
    halves = []
    for h in range(2):
        rows = (np.arange(TROWS) + h * HALFR - PAD) % H
        th = xt[:, rows]                       # (C, 128, 230, B)
        halves.append(np.transpose(th, (0, 1, 3, 2)))  # (C, 128, B, 230)
    xp = np.stack(halves, axis=1).astype(ml_dtypes.bfloat16)  # (C, 2, 128, B, 230)
    return [np.ascontiguousarray(xp[k * CPC : (k + 1) * CPC]) for k in range(NCORES)]


def _host_pack_w(kernel):
    """kernel (C,K,K) -> per-core banded lhsT [PROWS, CPC, K, HALFR] bf16.

    lhsT[p, cl, v, m] = wf[c0+cl, p-m, v] for 0 <= p-m < 7, else 0.
    """
    import ml_dtypes

    wf = np.ascontiguousarray(kernel[:, ::-1, ::-1]).astype(np.float32)  # (C, K, K)
    blobs = []
    m_idx = np.arange(HALFR)
    for k in range(NCORES):
        warr = np.zeros((TROWS, CPC, K, WCOLS), dtype=np.float32)
        wc = wf[k * CPC : (k + 1) * CPC]  # (24, 7, 7)
        for u in range(K):
            # warr[m+u, :, v, m] = wc[:, u, v]
            warr[m_idx + u, :, :, m_idx] = wc[:, u, :]
        blobs.append(warr.astype(ml_dtypes.bfloat16))
    return blobs


_NC_CACHE = {}


def _get_nc():
    if "nc" not in _NC_CACHE:
        nc = build_nc()
        nc.finalize()
        _NC_CACHE["nc"] = nc
    return _NC_CACHE["nc"]


def run(x, kernel, trace=False, **kw):
    assert x.shape == (B, H, W, C) and kernel.shape == (C, K, K)
    nc = _get_nc()
    xs = _host_pack_x(np.asarray(x).astype(np.float32))
    ws = _host_pack_w(np.asarray(kernel))
    in_maps = [{"x": xs[k], "w": ws[k]} for k in range(NCORES)]
    res = run_bass_kernel_spmd(nc, in_maps, list(range(NCORES)), trace=trace, **kw)
    # out blob [CPC, 2, HALFR, NQ, 448] -> (B, H, W, CPC) per core
    parts = []
    for k in range(NCORES):
        o = np.asarray(res.results[k]["out"]).astype(np.float32)
        o = o.reshape(CPC, 2, HALFR, B, W)          # (c, h, m, img, j)
        o = np.transpose(o, (3, 1, 2, 4, 0))        # (img, h, m, j, c)
        parts.append(o.reshape(B, H, W, CPC))
    out = np.concatenate(parts, axis=3)
    return np.ascontiguousarray(out), res


def kernel(x, kernel):
    out, _ = run(np.asarray(x), np.asarray(kernel))
    return out


# revision 23
# speedup vs baseline: 1.0159x; 1.0082x over previous
"""FFT depthwise conv == direct 7x7 circular depthwise conv, on 8 TRN2 cores.

out[b,i,j,c] = sum_{u,v} wf[c,u,v] * x[b,(i+u-3)%H,(j+v-3)%W,c],  wf = kernel[:, ::-1, ::-1]

Banded-matmul scheme: image ROWS live on SBUF partitions, so one matmul with a
7-diagonal banded stationary matrix covers all 7 row-taps (u) at once; the 7
column-taps (v) become 7 PSUM-accumulated matmuls whose rhs is the same tile
shifted by v columns.  49 taps in 7 matmuls instead of 49.

Sharding: channels (192/8 = 24 per core), so all 8 images stream through each
banded weight while it is stationary (weight-load overhead amortized 8x).

Per (channel, row-half): input tile [118 rows x (8 img x 230 cols)] bf16; for
v in 0..6: one matmul per image-pair q (PSUM bank q, [112 x 448] f32,
start=(v==0), stop=(v==6)).  lhsT[p, m] = wf[c, p-m, v] (7-diag band).
PSUM -> bf16 SBUF evac split across ScalarE/VectorE, one 401KB output DMA per
(c, half).  Host pre-builds the circularly-padded row-major tiles and banded
weights; host also reassembles the final (B,H,W,C) output.
"""

import os
import sys

for _p in ("/opt/trn_rl_repo", "/root/.axon_site/_ro/trn_rl_repo"):
    if os.path.isdir(_p) and _p not in sys.path:
        sys.path.insert(0, _p)

import numpy as np

import concourse.bacc as bacc
import concourse.bass as bass
import concourse.mybir as mybir
from concourse.bass_utils import run_bass_kernel_spmd
from concourse.tile import TileContext

F32 = mybir.dt.float32
BF16 = mybir.dt.bfloat16

B, H, W, C, K = 8, 224, 224, 192, 7
NCORES = 8
CPC = C // NCORES        # 24 channels per core
PAD = K // 2             # 3
HALFR = H // 2           # 112 output rows per half
PROWS = HALFR + 2 * PAD  # 118 input rows actually used per half-tile
TROWS = 128              # tile partition rows (16-multiple so the DMA
                         # descriptor balancer sprays all 16 SDMA engines;
                         # 118 partitions degenerate to a 2-engine split)
WCOLS = 128              # banded lhsT padded to 128x128 so walrus enables
                         # FWL (fast weight load, 2 bf16/cycle) -- the
                         # per-matmul LDWEIGHTS then hides under the stream
NCOL = W + 2 * PAD       # 230 padded cols
NQ = 4                   # image pairs per (c, half): 4 x 2 = 8 images
QCOLS = 2 * W            # 448 psum cols per pair
PREFETCH = 3


def build_nc():
    nc = bacc.Bacc()
    x_d = nc.declare_dram_parameter("x", [CPC, 2, TROWS, B, NCOL], BF16, isOutput=False)
    w_d = nc.declare_dram_parameter("w", [TROWS, CPC, K, WCOLS], BF16, isOutput=False)
    out_d = nc.declare_dram_parameter("out", [CPC, 2, HALFR, NQ, QCOLS], BF16, isOutput=True)

    mult = mybir.AluOpType.mult

    with TileContext(nc) as tc:
        with (
            tc.tile_pool(name="wp", bufs=1) as wpool,
            tc.tile_pool(name="xp", bufs=PREFETCH + 2) as xpool,
            tc.tile_pool(name="op", bufs=3) as opool,
            tc.tile_pool(name="pp", bufs=8, space="PSUM") as ppool,
        ):
            wsb = wpool.tile([TROWS, CPC, K, WCOLS], BF16)

            units = [(c, h) for c in range(CPC) for h in range(2)]
            pending = {}

            def issue_x(i):
                c, h = units[i]
                xt = xpool.tile([TROWS, B, NCOL], BF16, name=f"xt{c}_{h}", tag="xt")
                nc.sync.dma_start(out=xt[:], in_=x_d[c, h])
                pending[i] = xt

            def issue_w(c):
                # per-channel weight DMAs on the scalar queue (which carries
                # no activation-table load -- evac is DVE-only -- so these
                # start immediately after the prologue), issued lazily ~2
                # channels ahead of use
                nc.scalar.dma_start(out=wsb[:, c : c + 1], in_=w_d[:, c : c + 1])

            issue_w(0)
            issue_x(0)
            issue_w(1)
            for j in range(1, PREFETCH):
                issue_x(j)

            # ~64 dummy matmuls on never-written tiles (no deps): they run
            # during the ~5us input-DMA wait right after the prologue and
            # trip the PE HAM activity window, so the real matmul stream
            # starts at 2.4 GHz instead of paying the ~4us cold-ramp.
            warm_w = wpool.tile([TROWS, WCOLS], BF16, name="warmw", tag="warmw")
            warm_x = wpool.tile([TROWS, 64], BF16, name="warmx", tag="warmx")
            nc.gpsimd.memset(warm_w[:], 0)
            nc.gpsimd.memset(warm_x[:], 0)
            # same tag as the real psum tiles: occupies one rotating bank
            # slot (PSUM has exactly 8 banks; a 9th slot cannot exist)
            warm_ps = ppool.tile([TROWS, 64], F32, name="warmps", tag="ps")
            for _ in range(64):
                nc.tensor.matmul(warm_ps[:], warm_w[:], warm_x[:], start=True, stop=True)

            for i, (c, h) in enumerate(units):
                if i + PREFETCH < len(units):
                    issue_x(i + PREFETCH)
                if h == 0 and c + 2 < CPC:
                    issue_w(c + 2)
                xt = pending.pop(i)
                last = i == len(units) - 1
                # last unit: 8 single-image PSUM banks so evacuation and the
                # output DMA pipeline against the final matmuls (shorter tail)
                nq = 8 if last else NQ
                ncols = B * W // nq
                ipb = B // nq  # images per bank
                pss = []
                for q in range(nq):
                    ps = ppool.tile([TROWS, ncols], F32, name=f"ps{c}_{h}_{q}", tag="ps")
                    pss.append(ps)
                for v in range(K):
                    wap = wsb[:, c, v, :]
                    for q in range(nq):
                        nc.tensor.matmul(
                            pss[q][:],
                            wap,
                            xt[:, ipb * q : ipb * (q + 1), v : v + W],
                            start=(v == 0),
                            stop=(v == K - 1),
                        )
                ot = opool.tile([HALFR, nq, ncols], BF16, name=f"ot{c}_{h}", tag="ot")
                # evac on DVE only: using ScalarE would pull in a ~2.5us
                # InstLoadActFuncSet that head-blocks the scalar DMA queue
                for q in range(nq):
                    nc.vector.tensor_scalar(ot[:, q, :], pss[q][0:HALFR, :], 1.0, None, mult)
                    if last and q == nq // 2 - 1:
                        # first half of the final output leaves while the
                        # last banks are still accumulating/evacuating
                        nc.scalar.dma_start(
                            out=out_d[c, h, :, 0 : NQ // 2], in_=ot[:, 0 : nq // 2]
                        )
                if last:
                    nc.sync.dma_start(
                        out=out_d[c, h, :, NQ // 2 : NQ], in_=ot[:, nq // 2 : nq]
                    )
                else:
                    eng = nc.scalar if i % 2 == 0 else nc.sync
                    eng.dma_start(out=out_d[c, h], in_=ot[:])
    return nc


def _host_pack_x(x):
    """x (B,H,W,C) f32 -> per-core [CPC, 2, TROWS, B, NCOL] bf16."""
    import ml_dtypes

    xt = np.transpose(x, (3, 1, 2, 0))  # (C, H, W, B)
    xt = np.concatenate([xt[:, :, -PAD:, :], xt, xt[:, :, :PAD, :]], axis=2)  # (C,H,230,B)
    halves = []
    for h in range(2):
        rows = (np.arange(TROWS) + h * HALFR - PAD) % H
        th = xt[:, rows]                       # (C, 128, 230, B)
        halves.append(np.transpose(th, (0, 1, 3, 2)))  # (C, 128, B, 230)
    xp = np.stack(halves, axis=1).astype(ml_dtypes.bfloat16)  # (C, 2, 128, B, 230)
    return [np.ascontiguousarray(xp[k * CPC : (k + 1) * CPC]) for k in range(NCORES)]


def _host_pack_w(kernel):
    """kernel (C,K,K) -> per-core banded lhsT [PROWS, CPC, K, HALFR] bf16.

    lhsT[p, cl, v, m] = wf[c0+cl, p-m, v] for 0 <= p-m < 7, else 0.
    """
    import ml_dtypes

    wf = np.ascontiguousarray(kernel[:, ::-1, ::-1]).astype(np.float32)  # (C, K, K)
    blobs = []
    m_idx = np.arange(HALFR)
    for k in range(NCORES):
        warr = np.zeros((TROWS, CPC, K, WCOLS), dtype=np.float32)
        wc = wf[k * CPC : (k + 1) * CPC]  # (24, 7, 7)
        for u in range(K):
            # warr[m+u, :, v, m] = wc[:, u, v]
            warr[m_idx + u, :, :, m_idx] = wc[:, u, :]
        blobs.append(warr.astype(ml_dtypes.bfloat16))
    return blobs


_NC_CACHE = {}


def _get_nc():
    if "nc" not in _NC_CACHE:
        nc = build_nc()
        nc.finalize()
        _NC_CACHE["nc"] = nc
    return _NC_CACHE["nc"]


def run(x, kernel, trace=False, **kw):
    assert x.shape == (B, H, W, C) and kernel.shape == (C, K, K)
    nc = _get_nc()
    xs = _host_pack_x(np.asarray(x).astype(np.float32))
    ws = _host_pack_w(np.asarray(kernel))
    in_maps = [{"x": xs[k], "w": ws[k]} for k in range(NCORES)]
    res = run_bass_kernel_spmd(nc, in_maps, list(range(NCORES)), trace=trace, **kw)
    # out blob [CPC, 2, HALFR, NQ, 448] -> (B, H, W, CPC) per core
    parts = []
    for k in range(NCORES):
        o = np.asarray(res.results[k]["out"]).astype(np.float32)
        o = o.reshape(CPC, 2, HALFR, B, W)          # (c, h, m, img, j)
        o = np.transpose(o, (3, 1, 2, 4, 0))        # (img, h, m, j, c)
        parts.append(o.reshape(B, H, W, CPC))
    out = np.concatenate(parts, axis=3)
    return np.ascontiguousarray(out), res


def kernel(x, kernel):
    out, _ = run(np.asarray(x), np.asarray(kernel))
    return out


# revision 27
# speedup vs baseline: 1.0206x; 1.0047x over previous
"""FFT depthwise conv == direct 7x7 circular depthwise conv, on 8 TRN2 cores.

out[b,i,j,c] = sum_{u,v} wf[c,u,v] * x[b,(i+u-3)%H,(j+v-3)%W,c],  wf = kernel[:, ::-1, ::-1]

Banded-matmul scheme: image ROWS live on SBUF partitions, so one matmul with a
7-diagonal banded stationary matrix covers all 7 row-taps (u) at once; the 7
column-taps (v) become 7 PSUM-accumulated matmuls whose rhs is the same tile
shifted by v columns.  49 taps in 7 matmuls instead of 49.

Sharding: channels (192/8 = 24 per core), so all 8 images stream through each
banded weight while it is stationary (weight-load overhead amortized 8x).

Per (channel, row-half): input tile [118 rows x (8 img x 230 cols)] bf16; for
v in 0..6: one matmul per image-pair q (PSUM bank q, [112 x 448] f32,
start=(v==0), stop=(v==6)).  lhsT[p, m] = wf[c, p-m, v] (7-diag band).
PSUM -> bf16 SBUF evac split across ScalarE/VectorE, one 401KB output DMA per
(c, half).  Host pre-builds the circularly-padded row-major tiles and banded
weights; host also reassembles the final (B,H,W,C) output.
"""

import os
import sys

for _p in ("/opt/trn_rl_repo", "/root/.axon_site/_ro/trn_rl_repo"):
    if os.path.isdir(_p) and _p not in sys.path:
        sys.path.insert(0, _p)

import numpy as np

import concourse.bacc as bacc
import concourse.bass as bass
import concourse.mybir as mybir
from concourse.bass_utils import run_bass_kernel_spmd
from concourse.tile import TileContext

F32 = mybir.dt.float32
BF16 = mybir.dt.bfloat16

B, H, W, C, K = 8, 224, 224, 192, 7
NCORES = 8
CPC = C // NCORES        # 24 channels per core
PAD = K // 2             # 3
HALFR = H // 2           # 112 output rows per half
PROWS = HALFR + 2 * PAD  # 118 input rows actually used per half-tile
TROWS = 128              # tile partition rows (16-multiple so the DMA
                         # descriptor balancer sprays all 16 SDMA engines;
                         # 118 partitions degenerate to a 2-engine split)
WCOLS = 128              # banded lhsT padded to 128x128 so walrus enables
                         # FWL (fast weight load, 2 bf16/cycle) -- the
                         # per-matmul LDWEIGHTS then hides under the stream
NCOL = W + 2 * PAD       # 230 padded cols
NQ = 4                   # image pairs per (c, half): 4 x 2 = 8 images
QCOLS = 2 * W            # 448 psum cols per pair
PREFETCH = 3


def build_nc():
    nc = bacc.Bacc()
    x_d = nc.declare_dram_parameter("x", [CPC, 2, TROWS, B, NCOL], BF16, isOutput=False)
    w_d = nc.declare_dram_parameter("w", [TROWS, CPC, K, WCOLS], BF16, isOutput=False)
    out_d = nc.declare_dram_parameter("out", [CPC, 2, HALFR, NQ, QCOLS], BF16, isOutput=True)

    mult = mybir.AluOpType.mult

    with TileContext(nc) as tc:
        with (
            tc.tile_pool(name="wp", bufs=1) as wpool,
            tc.tile_pool(name="xp", bufs=PREFETCH + 2) as xpool,
            tc.tile_pool(name="op", bufs=3) as opool,
            tc.tile_pool(name="pp", bufs=8, space="PSUM") as ppool,
        ):
            wsb = wpool.tile([TROWS, CPC, K, WCOLS], BF16)

            units = [(c, h) for c in range(CPC) for h in range(2)]
            pending = {}

            def issue_x(i):
                c, h = units[i]
                xt = xpool.tile([TROWS, B, NCOL], BF16, name=f"xt{c}_{h}", tag="xt")
                nc.sync.dma_start(out=xt[:], in_=x_d[c, h])
                pending[i] = xt

            def issue_w(c):
                # per-channel weight DMAs on the scalar queue (which carries
                # no activation-table load -- evac is DVE-only -- so these
                # start immediately after the prologue), issued lazily ~2
                # channels ahead of use
                nc.scalar.dma_start(out=wsb[:, c : c + 1], in_=w_d[:, c : c + 1])

            issue_w(0)
            issue_x(0)
            issue_w(1)
            for j in range(1, PREFETCH):
                issue_x(j)

            # ~64 dummy matmuls on never-written tiles (no deps): they run
            # during the ~5us input-DMA wait right after the prologue and
            # trip the PE HAM activity window, so the real matmul stream
            # starts at 2.4 GHz instead of paying the ~4us cold-ramp.
            warm_w = wpool.tile([TROWS, WCOLS], BF16, name="warmw", tag="warmw")
            warm_x = wpool.tile([TROWS, 64], BF16, name="warmx", tag="warmx")
            nc.gpsimd.memset(warm_w[:], 0)
            nc.gpsimd.memset(warm_x[:], 0)
            # same tag as the real psum tiles: occupies one rotating bank
            # slot (PSUM has exactly 8 banks; a 9th slot cannot exist)
            warm_ps = ppool.tile([TROWS, 64], F32, name="warmps", tag="ps")
            for _ in range(64):
                nc.tensor.matmul(warm_ps[:], warm_w[:], warm_x[:], start=True, stop=True)

            for i, (c, h) in enumerate(units):
                if i + PREFETCH < len(units):
                    issue_x(i + PREFETCH)
                if h == 0 and c + 2 < CPC:
                    issue_w(c + 2)
                xt = pending.pop(i)
                last = i == len(units) - 1
                # last unit: 8 single-image PSUM banks so evacuation and the
                # output DMA pipeline against the final matmuls (shorter tail)
                nq = 8 if last else NQ
                ncols = B * W // nq
                ipb = B // nq  # images per bank
                pss = []
                for q in range(nq):
                    ps = ppool.tile([TROWS, ncols], F32, name=f"ps{c}_{h}_{q}", tag="ps")
                    pss.append(ps)
                for v in range(K):
                    wap = wsb[:, c, v, :]
                    for q in range(nq):
                        nc.tensor.matmul(
                            pss[q][:],
                            wap,
                            xt[:, ipb * q : ipb * (q + 1), v : v + W],
                            start=(v == 0),
                            stop=(v == K - 1),
                        )
                ot = opool.tile([HALFR, nq, ncols], BF16, name=f"ot{c}_{h}", tag="ot")
                # evac on DVE only: using ScalarE would pull in a ~2.5us
                # InstLoadActFuncSet that head-blocks the scalar DMA queue
                for q in range(nq):
                    nc.vector.tensor_scalar(ot[:, q, :], pss[q][0:HALFR, :], 1.0, None, mult)
                    if last and q == nq // 2 - 1:
                        # first half of the final output leaves while the
                        # last banks are still accumulating/evacuating
                        nc.scalar.dma_start(
                            out=out_d[c, h, :, 0 : NQ // 2], in_=ot[:, 0 : nq // 2]
                        )
                if last:
                    nc.sync.dma_start(
                        out=out_d[c, h, :, NQ // 2 : NQ], in_=ot[:, nq // 2 : nq]
                    )
                else:
                    eng = nc.scalar if i % 2 == 0 else nc.sync
                    eng.dma_start(out=out_d[c, h], in_=ot[:])
    return nc


def _host_pack_x(x):
    """x (B,H,W,C) f32 -> per-core [CPC, 2, TROWS, B, NCOL] bf16."""
    import ml_dtypes

    xt = np.transpose(x, (3, 1, 2, 0))  # (C, H, W, B)
    xt = np.concatenate([xt[:, :, -PAD:, :], xt, xt[:, :, :PAD, :]], axis=2)  # (C,H,230,B)
    halves = []
    for h in range(2):
        rows = (np.arange(TROWS) + h * HALFR - PAD) % H
        th = xt[:, rows]                       # (C, 128, 230, B)
        halves.append(np.transpose(th, (0, 1, 3, 2)))  # (C, 128, B, 230)
    xp = np.stack(halves, axis=1).astype(ml_dtypes.bfloat16)  # (C, 2, 128, B, 230)
    return [np.ascontiguousarray(xp[k * CPC : (k + 1) * CPC]) for k in range(NCORES)]


def _host_pack_w(kernel):
    """kernel (C,K,K) -> per-core banded lhsT [PROWS, CPC, K, HALFR] bf16.

    lhsT[p, cl, v, m] = wf[c0+cl, p-m, v] for 0 <= p-m < 7, else 0.
    """
    import ml_dtypes

    wf = np.ascontiguousarray(kernel[:, ::-1, ::-1]).astype(np.float32)  # (C, K, K)
    blobs = []
    m_idx = np.arange(HALFR)
    for k in range(NCORES):
        warr = np.zeros((TROWS, CPC, K, WCOLS), dtype=np.float32)
        wc = wf[k * CPC : (k + 1) * CPC]  # (24, 7, 7)
        for u in range(K):
            # warr[m+u, :, v, m] = wc[:, u, v]
            warr[m_idx + u, :, :, m_idx] = wc[:, u, :]
        blobs.append(warr.astype(ml_dtypes.bfloat16))
    return blobs


_NC_CACHE = {}


def _get_nc():
    if "nc" not in _NC_CACHE:
        nc = build_nc()
        nc.finalize()
        _NC_CACHE["nc"] = nc
    return _NC_CACHE["nc"]


def run(x, kernel, trace=False, **kw):
    assert x.shape == (B, H, W, C) and kernel.shape == (C, K, K)
    nc = _get_nc()
    xs = _host_pack_x(np.asarray(x).astype(np.float32))
    ws = _host_pack_w(np.asarray(kernel))
    in_maps = [{"x": xs[k], "w": ws[k]} for k in range(NCORES)]
    res = run_bass_kernel_spmd(nc, in_maps, list(range(NCORES)), trace=trace, **kw)
    # out blob [CPC, 2, HALFR, NQ, 448] -> (B, H, W, CPC) per core
    parts = []
    for k in range(NCORES):
        o = np.asarray(res.results[k]["out"]).astype(np.float32)
        o = o.reshape(CPC, 2, HALFR, B, W)          # (c, h, m, img, j)
        o = np.transpose(o, (3, 1, 2, 4, 0))        # (img, h, m, j, c)
        parts.append(o.reshape(B, H, W, CPC))
    out = np.concatenate(parts, axis=3)
    return np.ascontiguousarray(out), res


def kernel(x, kernel):
    out, _ = run(np.asarray(x), np.asarray(kernel))
    return out
